# revision 2
# baseline (speedup 1.0000x reference)
"""Trainium2 Bass kernel for EntityAttention.

    beta[b,e,a] = (agent[b,e] @ w_psi) . (vis[b,e,a] @ w_phi)
    out         = softmax_a(beta)

Refactored so the huge `visible_observations` tensor is only read once, in
its natural layout, by a fused multiply+reduce on the Vector engine:

    qT[k, be]   = sum_din w_psi[din, k] * agent[be, din]      (PE)
    t[be, dout] = sum_k   qT[k, be]     * w_phiT[k, dout]     (PE)
    beta[be, a] = sum_d   vis[be, a, d] * t[be, d]            (DVE TTR)
    out[be, a]  = softmax_a(beta)                             (DVE + ACT)

Sharding: data-parallel over the batch axis across 8 NeuronCores
(16 batches / core); w_psi / w_phi replicated.
"""

from contextlib import ExitStack

import numpy as np

import concourse.bass as bass  # noqa: F401  (bass types used via tile/bacc)
import concourse.tile as tile
from concourse import bacc, bass_utils, mybir
from concourse.masks import make_identity

# Problem shape (hardcoded per contract; kernel.py must be self-contained).
B, E, A, D, K = 128, 32, 16, 512, 128
N_CORES = 8
B_SH = B // N_CORES          # batches per core = 16
BE = B_SH * E                # rows per core = 512
NBC = BE // 128              # be-chunks of 128 partitions = 4
NDC = D // 128               # din-chunks = 4
HALF_A = 8                   # visible-agents per streamed half tile
F32 = mybir.dt.float32


def _emit(tc, nc, ag_d, vis_d, wpsi_d, wphi_d, out_d):
    with ExitStack() as ctx:
        const = ctx.enter_context(tc.tile_pool(name="const", bufs=1))
        agp = ctx.enter_context(tc.tile_pool(name="agp", bufs=2))
        visp = ctx.enter_context(tc.tile_pool(name="visp", bufs=3))
        small = ctx.enter_context(tc.tile_pool(name="small", bufs=4))
        ps_tr = ctx.enter_context(tc.tile_pool(name="ps_tr", bufs=3, space="PSUM"))
        ps_mm = ctx.enter_context(tc.tile_pool(name="ps_mm", bufs=2, space="PSUM"))

        ident = const.tile([128, 128], F32)
        make_identity(nc, ident)

        # Weights, natural layout: w_sb[p, i, k] = w[i*128 + p, k]
        wpsi_sb = const.tile([128, NDC, K], F32)
        nc.sync.dma_start(out=wpsi_sb, in_=wpsi_d.rearrange("(i p) k -> p i k", p=128))
        wphi_sb = const.tile([128, NDC, K], F32)
        nc.sync.dma_start(out=wphi_sb, in_=wphi_d.rearrange("(j p) k -> p j k", p=128))

        # w_phiT[k, j, dout_local] = w_phi[j*128 + dout_local, k]
        wphiT_sb = const.tile([128, NDC, 128], F32)
        for j in range(NDC):
            tr_ps = ps_tr.tile([128, 128], F32, tag="tr", name=f"trw{j}")
            nc.tensor.transpose(tr_ps, wphi_sb[:, j, :], ident)
            nc.scalar.copy(wphiT_sb[:, j, :], tr_ps)

        # agT[p, i, be] = agent[be, i*128 + p]
        agT_sb = const.tile([128, NDC, BE], F32)
        qT_sb = const.tile([128, BE], F32)
        t_tiles = []
        for c in range(NBC):
            cs = slice(c * 128, (c + 1) * 128)
            ag_sb = agp.tile([128, D], F32, tag="ag", name=f"ag{c}")
            nc.sync.dma_start(out=ag_sb, in_=ag_d[cs, :])
            for i in range(NDC):
                tr_ps = ps_tr.tile([128, 128], F32, tag="tr", name=f"tra{c}_{i}")
                nc.tensor.transpose(tr_ps, ag_sb[:, i * 128:(i + 1) * 128], ident)
                nc.scalar.copy(agT_sb[:, i, cs], tr_ps)
            # qT[:, cs] = sum_i w_psi_chunk_i.T @ agT_chunk_i
            qt_ps = ps_mm.tile([128, 128], F32, tag="qt", name=f"qt{c}")
            for i in range(NDC):
                nc.tensor.matmul(
                    qt_ps,
                    lhsT=wpsi_sb[:, i, :],
                    rhs=agT_sb[:, i, cs],
                    start=(i == 0),
                    stop=(i == NDC - 1),
                )
            nc.scalar.copy(qT_sb[:, cs], qt_ps)
            # t[be_c, dout] = qT[:, cs].T @ w_phiT
            t_ps = ps_mm.tile([128, D], F32, tag="t", name=f"tps{c}")
            nc.tensor.matmul(
                t_ps, lhsT=qT_sb[:, cs], rhs=wphiT_sb[:, :, :], start=True, stop=True
            )
            t_sb = const.tile([128, D], F32, tag=f"t{c}", name=f"t{c}")
            nc.scalar.copy(t_sb, t_ps)
            t_tiles.append(t_sb)

        # Stream visible; fused multiply+reduce into beta; softmax over a.
        for c in range(NBC):
            cs = slice(c * 128, (c + 1) * 128)
            beta_sb = small.tile([128, A], F32, tag="beta", name=f"beta{c}")
            for h in range(A // HALF_A):
                vis_sb = visp.tile([128, HALF_A, D], F32, tag="vis", name=f"vis{c}_{h}")
                nc.sync.dma_start(
                    out=vis_sb,
                    in_=vis_d[cs, h * HALF_A * D:(h + 1) * HALF_A * D],
                )
                for al in range(HALF_A):
                    a = h * HALF_A + al
                    dummy = small.tile([128, 1], F32, tag="ttr_dummy", name=f"dm{c}_{a}")
                    # out = (in0*1 + 0) * in1 ; accum_out = sum(out)
                    nc.vector.affine_mul_reduce(
                        out=dummy.broadcast_to((128, D)),
                        accum_out=beta_sb[:, a:a + 1],
                        in0=vis_sb[:, al, :],
                        in1=t_tiles[c],
                        scale=1.0,
                        bias=0.0,
                    )
            negm = small.tile([128, 1], F32, tag="negm", name=f"negm{c}")
            nc.vector.tensor_reduce(
                negm, beta_sb, axis=mybir.AxisListType.X,
                op=mybir.AluOpType.max, negate=True,
            )
            prob = small.tile([128, A], F32, tag="prob", name=f"prob{c}")
            ssum = small.tile([128, 1], F32, tag="ssum", name=f"ssum{c}")
            nc.scalar.activation(
                prob, beta_sb, mybir.ActivationFunctionType.Exp,
                bias=negm, scale=1.0, accum_out=ssum,
            )
            rec = small.tile([128, 1], F32, tag="rec", name=f"rec{c}")
            nc.vector.reciprocal(rec, ssum)
            osb = small.tile([128, A], F32, tag="osb", name=f"osb{c}")
            nc.vector.tensor_scalar_mul(osb, prob, rec)
            nc.sync.dma_start(out=out_d[cs, :], in_=osb)


def _build_program():
    nc = bacc.Bacc("TRN2", target_bir_lowering=False, debug=False)
    ag_d = nc.dram_tensor("agent", (BE, D), F32, kind="ExternalInput").ap()
    vis_d = nc.dram_tensor("vis", (BE, A * D), F32, kind="ExternalInput").ap()
    wpsi_d = nc.dram_tensor("w_psi", (D, K), F32, kind="ExternalInput").ap()
    wphi_d = nc.dram_tensor("w_phi", (D, K), F32, kind="ExternalInput").ap()
    out_d = nc.dram_tensor("out", (BE, A), F32, kind="ExternalOutput").ap()
    with tile.TileContext(nc) as tc:
        _emit(tc, nc, ag_d, vis_d, wpsi_d, wphi_d, out_d)
    nc.compile()
    return nc


_PROG = None


def _get_program():
    global _PROG
    if _PROG is None:
        _PROG = _build_program()
    return _PROG


def make_in_maps(agent_observation, visible_observations, w_psi, w_phi):
    agent = np.ascontiguousarray(np.asarray(agent_observation, np.float32)).reshape(B, E, D)
    vis = np.ascontiguousarray(np.asarray(visible_observations, np.float32)).reshape(B, E, A, D)
    wpsi = np.ascontiguousarray(np.asarray(w_psi, np.float32))
    wphi = np.ascontiguousarray(np.asarray(w_phi, np.float32))
    in_maps = []
    for ci in range(N_CORES):
        sl = slice(ci * B_SH, (ci + 1) * B_SH)
        in_maps.append({
            "agent": np.ascontiguousarray(agent[sl].reshape(BE, D)),
            "vis": np.ascontiguousarray(vis[sl].reshape(BE, A * D)),
            "w_psi": wpsi,
            "w_phi": wphi,
        })
    return in_maps


def run_sharded(in_maps, trace=False, **kwargs):
    nc = _get_program()
    return bass_utils.run_bass_kernel_spmd(
        nc, in_maps, core_ids=list(range(N_CORES)), trace=trace, **kwargs
    )


def kernel(agent_observation, visible_observations, w_psi, w_phi):
    in_maps = make_in_maps(agent_observation, visible_observations, w_psi, w_phi)
    res = run_sharded(in_maps)
    return np.concatenate(
        [r["out"].reshape(B_SH, E, A) for r in res.results], axis=0
    )
